# revision 15
# baseline (speedup 1.0000x reference)
"""GPT3 dev block (B=2,S=2048,D=1024,H=16,DH=64,FF=4096) on 8 trn2 NeuronCores.

Sharding: data-parallel over batch (2 groups of 4 cores) x 4-way tensor
parallel, per the Megatron hint, but with AllGather/ReduceScatter instead of
AllReduce (half the wire bytes):

  per core (group g = c//4 owns batch g, rank r = c%4):
    - activations transposed on-chip [feature(partition), row(free)] so every
      GEMM contracts on the partition axis with weights in natural layout.
    - LayerNorm folded into the next GEMM:
        W^T((x-mu)*rstd) = rstd[s] * (W^T x - mu[s]*colsum(W))
      -> GEMM runs on RAW x; a K=1 rank-one matmul adds -mu[s]*colsum(W) into
      PSUM; rstd[s] broadcast multiply happens in the PSUM-eviction op.
    - QKV for 4 local heads; flash-style attention; softmax denominator from a
      ones-column appended to V in the PV matmul; no max subtraction (scores
      are O(1) at this scale).
    - attention heads AllGather'd (2 chunks) -> every rank holds the full
      context, computes a 256-feature slice of c_proj with FULL contraction
      (no reduction) + residual -> h2 slice; h2 AllGather'd (2 chunks).
    - MLP Megatron-split over FF (1024/rank), LN2 folded as above, erf-Gelu on
      ACT; proj partials ReduceScatter'd in 4 chunks with w_proj columns
      permuted host-side so each rank receives exactly its own 256 output
      features; + residual -> output slice written transposed [256, 2048].
  All arithmetic fp32 (PE fp32 matmul streams at the same cycles/column as
  bf16 on trn2, so fp32 costs nothing extra on the tensor engine).
"""

from contextlib import ExitStack

import numpy as np

B, S, D = 2, 2048, 1024
H, DH = 16, 64
FF = 4 * D
EPS = 1e-5
SCALE = 1.0 / 8.0

NCORES = 8
RANKS = 4
GROUPS = [[0, 1, 2, 3], [4, 5, 6, 7]]
DR = D // RANKS          # 256 per-rank feature slice
FR = FF // RANKS         # 1024 per-rank ff slice
HLOC = H // RANKS        # 4 local heads
NT = S // 128            # 16 key tiles
NC4 = S // 512           # 4 column chunks
KB = D // 128            # 8 contraction blocks over D
MF = FR // 128           # 8 ff blocks

_CACHE = {}


def _build(with_biases):
    from concourse import bacc, mybir, tile
    from concourse.masks import make_identity

    f32 = mybir.dt.float32
    AF = mybir.ActivationFunctionType
    nc = bacc.Bacc("TRN2", target_bir_lowering=False, debug=False,
                   num_devices=NCORES)

    # ---- I/O (identical names/shapes on every core; data differs) ----
    xT_d = nc.dram_tensor("xT", [D, S], f32, kind="ExternalInput")
    xselT_d = nc.dram_tensor("xselT", [DR, S], f32, kind="ExternalInput")
    wqkv_d = nc.dram_tensor("wqkv", [D, 3 * HLOC * DH], f32, kind="ExternalInput")
    wqkvsum_d = nc.dram_tensor("wqkvsum", [1, 3 * HLOC * DH], f32,
                               kind="ExternalInput")
    wao_d = nc.dram_tensor("wao", [D, DR], f32, kind="ExternalInput")
    wfc_d = nc.dram_tensor("wfc", [D, FR], f32, kind="ExternalInput")
    wfcsum_d = nc.dram_tensor("wfcsum", [1, FR], f32, kind="ExternalInput")
    wproj_d = nc.dram_tensor("wproj", [FR, D], f32, kind="ExternalInput")
    if with_biases:
        bqkv_d = nc.dram_tensor("bqkv", [3 * HLOC * DH, 1], f32,
                                kind="ExternalInput")
        bao_d = nc.dram_tensor("bao", [DR, 1], f32, kind="ExternalInput")
        bfc_d = nc.dram_tensor("bfc", [FR, 1], f32, kind="ExternalInput")
        bproj_d = nc.dram_tensor("bproj", [DR, 1], f32, kind="ExternalInput")
    out_d = nc.dram_tensor("out", [DR, S], f32, kind="ExternalOutput")

    PT = ["pa", "pb", "pc", "pd"]

    with tile.TileContext(nc) as tc, ExitStack() as ctx:
        const = ctx.enter_context(tc.tile_pool(name="const", bufs=1))
        big = ctx.enter_context(tc.tile_pool(name="big", bufs=1))
        med = ctx.enter_context(tc.tile_pool(name="med", bufs=1))
        stream = ctx.enter_context(tc.tile_pool(name="stream", bufs=2))
        ev = ctx.enter_context(tc.tile_pool(name="ev", bufs=4))
        ps = ctx.enter_context(tc.tile_pool(name="ps", bufs=2, space="PSUM"))
        dram = ctx.enter_context(tc.tile_pool(name="dram", bufs=1, space="DRAM"))

        # ---------- constants ----------
        ident = const.tile([128, 128], f32, tag="ident")
        make_identity(nc, ident[:])
        ones_r = const.tile([1, 128], f32, tag="ones_r")   # K=1 lhsT
        nc.vector.memset(ones_r[:], 1.0)
        ones_c = const.tile([128, 1], f32, tag="ones_c")   # partition-sum lhsT
        nc.vector.memset(ones_c[:], 1.0)
        eps_t = const.tile([1, 1], f32, tag="eps")
        nc.vector.memset(eps_t[:], EPS)

        wqkvsum = const.tile([1, 3 * HLOC * DH], f32, tag="wqkvsum")
        nc.sync.dma_start(wqkvsum[:], wqkvsum_d[:])
        wfcsum = const.tile([1, FR], f32, tag="wfcsum")
        nc.sync.dma_start(wfcsum[:], wfcsum_d[:])
        if with_biases:
            bqkv_sb = const.tile([128, 6, 1], f32, tag="bqkv")
            nc.sync.dma_start(bqkv_sb[:],
                              bqkv_d[:].rearrange("(m p) o -> p m o", p=128))
            bao_sb = const.tile([128, 2, 1], f32, tag="bao")
            nc.sync.dma_start(bao_sb[:],
                              bao_d[:].rearrange("(m p) o -> p m o", p=128))
            bfc_sb = const.tile([128, MF, 1], f32, tag="bfc")
            nc.sync.dma_start(bfc_sb[:],
                              bfc_d[:].rearrange("(m p) o -> p m o", p=128))
            bproj_sb = const.tile([128, 2, 1], f32, tag="bproj")
            nc.sync.dma_start(bproj_sb[:],
                              bproj_d[:].rearrange("(m p) o -> p m o", p=128))

        s1_row = const.tile([1, S], f32, tag="s1r")
        nmu1_row = const.tile([1, S], f32, tag="nmu1r")

        def ln_stats(src, s_row, nmu_row):
            """src [128, KB, S] raw transposed activation -> per-row
            s_row = 1/sqrt(var+eps), nmu_row = -mean over all KB*128 feats."""
            for c4 in range(NC4):
                sl = slice(512 * c4, 512 * (c4 + 1))
                sum_ps = ps.tile([1, 512], f32, tag="pa")
                ssq_ps = ps.tile([1, 512], f32, tag="pb")
                for kb in range(KB):
                    nc.tensor.matmul(sum_ps[:], ones_c[:], src[:, kb, sl],
                                     start=(kb == 0), stop=(kb == KB - 1))
                for kb in range(KB):
                    sq = ev.tile([128, 512], f32, tag="ev_a")
                    nc.vector.tensor_mul(sq[:], src[:, kb, sl], src[:, kb, sl])
                    nc.tensor.matmul(ssq_ps[:], ones_c[:], sq[:],
                                     start=(kb == 0), stop=(kb == KB - 1))
                nc.vector.tensor_scalar_mul(nmu_row[0:1, sl], sum_ps[:],
                                            -1.0 / D)
                msq = ev.tile([1, 512], f32, tag="ev_s")
                nc.vector.tensor_scalar_mul(msq[:], ssq_ps[:], 1.0 / D)
                mu2 = ev.tile([1, 512], f32, tag="ev_s")
                nc.vector.tensor_mul(mu2[:], nmu_row[0:1, sl], nmu_row[0:1, sl])
                var = ev.tile([1, 512], f32, tag="ev_s")
                nc.vector.tensor_sub(var[:], msq[:], mu2[:])
                nc.scalar.activation(out=var[:], in_=var[:], func=AF.Sqrt,
                                     bias=eps_t[0:1, 0:1])
                nc.vector.reciprocal(s_row[0:1, sl], var[:])

        def bcast_row(row, out_b):
            """out_b [128, S] = row [1, S] replicated (K=1 matmul)."""
            for c4 in range(NC4):
                sl = slice(512 * c4, 512 * (c4 + 1))
                bc = ps.tile([128, 512], f32, tag="pc")
                nc.tensor.matmul(bc[:], ones_r[0:1, :], row[0:1, sl],
                                 start=True, stop=True)
                nc.vector.tensor_copy(out_b[:, sl], bc[:])

        # ---------- phase 1: load xT, LN1 stats ----------
        xT = big.tile([128, KB, S], f32, tag="big_a")
        nc.sync.dma_start(xT[:], xT_d[:].rearrange("(kb p) s -> p kb s", p=128))
        ln_stats(xT, s1_row, nmu1_row)
        s1_b = med.tile([128, S], f32, tag="mid_d")
        bcast_row(s1_row, s1_b)

        # ---------- phase 2: QKV GEMM on raw xT (LN1 folded) ----------
        qkT = big.tile([128, 4, S], f32, tag="big_c")     # q blocks 0-1, k 2-3
        vT = med.tile([128, 2, S], f32, tag="mid_v")      # -> attnT -> xselT
        for m in range(6):
            wqkv_m = stream.tile([128, KB, 128], f32, tag="ctx_kb",
                                 name=f"wqkv_m{m}")
            nc.sync.dma_start(wqkv_m[:],
                              wqkv_d[:, 128 * m:128 * (m + 1)]
                              .rearrange("(kb p) m -> p kb m", p=128))
            accs = [ps.tile([128, 512], f32, tag=PT[c4], name=f"acc{c4}") for c4 in range(NC4)]
            for kb in range(KB):
                for c4 in range(NC4):
                    nc.tensor.matmul(
                        accs[c4][:], wqkv_m[:, kb, :],
                        xT[:, kb, 512 * c4:512 * (c4 + 1)],
                        start=(kb == 0), stop=False)
            for c4 in range(NC4):
                sl = slice(512 * c4, 512 * (c4 + 1))
                nc.tensor.matmul(accs[c4][:],
                                 wqkvsum[0:1, 128 * m:128 * (m + 1)],
                                 nmu1_row[0:1, sl], start=False, stop=True)
                dst = qkT[:, m, sl] if m < 4 else vT[:, m - 4, sl]
                nc.vector.tensor_mul(dst, accs[c4][:], s1_b[:, sl])
                if with_biases:
                    nc.vector.tensor_scalar_add(dst, dst, bqkv_sb[:, m, 0:1])

        # ---------- phase 3: v -> [t, dh] layout with ones column ----------
        v_td = med.tile([128, NT, HLOC, DH + 1], f32, tag="mid_b")
        nc.vector.memset(v_td[:, :, :, DH:DH + 1], 1.0)
        for h in range(HLOC):
            vrow = slice(64 * (h % 2), 64 * (h % 2) + 64)
            for t in range(NT):
                tp = ps.tile([128, 64], f32, tag=PT[t % 2])
                nc.tensor.transpose(tp[:], vT[vrow, h // 2,
                                              128 * t:128 * (t + 1)],
                                    ident[vrow, vrow])
                nc.vector.tensor_copy(v_td[:, t, h, 0:DH], tp[:])

        # ---------- phase 4: attention, heads processed in pair blocks ----------
        attnT = med.tile([128, 2, S], f32, tag="mid_v")   # reuses vT slot
        ag_in = [dram.tile([128, S], f32, tag=f"ag_in{p}", name=f"ag_in{p}") for p in range(2)]
        ag_out = [dram.tile([RANKS * 128, S], f32, tag=f"ag_out{p}",
                            name=f"ag_out{p}") for p in range(2)]
        for pb in range(2):             # local heads (2pb, 2pb+1)
            for sc in range(NC4):
                ssl = slice(512 * sc, 512 * (sc + 1))
                at_ps = [ps.tile([DH + 1, 512], f32, tag=PT[2 + hh],
                                  name=f"at_ps{hh}") for hh in range(2)]
                for t in range(NT):
                    tsl = slice(128 * t, 128 * (t + 1))
                    for hh in range(2):
                        hrow = slice(64 * hh, 64 * hh + 64)
                        sc_ps = ps.tile([128, 512], f32, tag=PT[hh])
                        nc.tensor.matmul(sc_ps[:], qkT[hrow, 2 + pb, tsl],
                                         qkT[hrow, pb, ssl],
                                         start=True, stop=True)
                        probs = ev.tile([128, 512], f32, tag="ev_a")
                        nc.scalar.activation(out=probs[:], in_=sc_ps[:],
                                             func=AF.Exp, scale=SCALE)
                        nc.tensor.matmul(at_ps[hh][:],
                                         v_td[:, t, 2 * pb + hh, :], probs[:],
                                         start=(t == 0), stop=(t == NT - 1))
                for hh in range(2):
                    recip = ev.tile([1, 512], f32, tag="ev_s")
                    nc.vector.reciprocal(recip[:], at_ps[hh][DH:DH + 1, :])
                    bc = ps.tile([64, 512], f32, tag=PT[hh])
                    nc.tensor.matmul(bc[:], ones_r[0:1, 0:64], recip[:],
                                     start=True, stop=True)
                    bc_sb = ev.tile([64, 512], f32, tag="ev_b")
                    nc.vector.tensor_copy(bc_sb[:], bc[:])
                    nc.vector.tensor_mul(attnT[64 * hh:64 * (hh + 1), pb, ssl],
                                         at_ps[hh][0:DH, :], bc_sb[:])
            nc.sync.dma_start(ag_in[pb][:], attnT[:, pb, :])
            nc.gpsimd.collective_compute(
                "AllGather", mybir.AluOpType.bypass, replica_groups=GROUPS,
                ins=[ag_in[pb][:].opt()], outs=[ag_out[pb][:].opt()])

        # ---------- phase 5: c_proj slice (full contraction) + residual ----------
        # context feature block kb = 2*rho + pb sits at ag_out[pb] rows
        # [128*rho, 128*rho+128)  (= global heads 4rho+2pb, +1 -> natural order)
        xselT = med.tile([128, 2, S], f32, tag="mid_v")    # reuses attnT slot
        nc.sync.dma_start(xselT[:],
                          xselT_d[:].rearrange("(m p) s -> p m s", p=128))
        wao_sb = med.tile([128, KB, DR], f32, tag="mid_d")  # reuses s1_b slot
        nc.sync.dma_start(wao_sb[:],
                          wao_d[:].rearrange("(kb p) m -> p kb m", p=128))
        h2s = med.tile([128, 2, S], f32, tag="mid_b")     # reuses v_td slot
        ag2_in = [dram.tile([128, S], f32, tag=f"ag2_in{m}", name=f"ag2_in{m}") for m in range(2)]
        ag2_out = [dram.tile([RANKS * 128, S], f32, tag=f"ag2_out{m}",
                             name=f"ag2_out{m}") for m in range(2)]
        for m in range(2):
            accs = [ps.tile([128, 512], f32, tag=PT[c4], name=f"acc{c4}") for c4 in range(NC4)]
            for rho in range(RANKS):
                for pb in range(2):
                    kb = 2 * rho + pb
                    ctx_kb = stream.tile([128, S], f32, tag="ctx_kb")
                    nc.sync.dma_start(ctx_kb[:],
                                      ag_out[pb][128 * rho:128 * (rho + 1), :])
                    for c4 in range(NC4):
                        nc.tensor.matmul(
                            accs[c4][:], wao_sb[:, kb, 128 * m:128 * (m + 1)],
                            ctx_kb[:, 512 * c4:512 * (c4 + 1)],
                            start=(kb == 0), stop=(kb == KB - 1))
            for c4 in range(NC4):
                sl = slice(512 * c4, 512 * (c4 + 1))
                nc.vector.tensor_add(h2s[:, m, sl], accs[c4][:],
                                     xselT[:, m, sl])
                if with_biases:
                    nc.vector.tensor_scalar_add(h2s[:, m, sl], h2s[:, m, sl],
                                                bao_sb[:, m, 0:1])
            nc.sync.dma_start(ag2_in[m][:], h2s[:, m, :])
            nc.gpsimd.collective_compute(
                "AllGather", mybir.AluOpType.bypass, replica_groups=GROUPS,
                ins=[ag2_in[m][:].opt()], outs=[ag2_out[m][:].opt()])

        # ---------- phase 6: h2_full + LN2 stats ----------
        h2_full = big.tile([128, KB, S], f32, tag="big_a")   # reuses xT slot
        for rho in range(RANKS):
            for m in range(2):
                nc.sync.dma_start(h2_full[:, 2 * rho + m, :],
                                  ag2_out[m][128 * rho:128 * (rho + 1), :])
        s2_row = const.tile([1, S], f32, tag="s1r")    # reuses s1_row slot
        nmu2_row = const.tile([1, S], f32, tag="nmu1r")
        ln_stats(h2_full, s2_row, nmu2_row)
        s2_b = med.tile([128, S], f32, tag="mid_d")          # reuses s1_b slot
        bcast_row(s2_row, s2_b)

        # ---------- phase 7: MLP over two s-halves ----------
        rs_in = [dram.tile([2 * 128, S], f32, tag=f"rs_in{q}", name=f"rs_in{q}") for q in range(4)]
        rs_out = [dram.tile([64, S], f32, tag=f"rs_out{q}", name=f"rs_out{q}") for q in range(4)]
        for sh in range(2):
            gT = big.tile([128, MF, S // 2], f32, tag="big_c")  # reuses qkT
            for fm in range(MF):
                wfc_fm = stream.tile([128, KB, 128], f32, tag="ctx_kb")
                nc.sync.dma_start(
                    wfc_fm[:], wfc_d[:, 128 * fm:128 * (fm + 1)]
                    .rearrange("(kb p) m -> p kb m", p=128))
                accs = [ps.tile([128, 512], f32, tag=PT[ch], name=f"facc{ch}") for ch in range(2)]
                for kb in range(KB):
                    for ch in range(2):
                        c4 = 2 * sh + ch
                        nc.tensor.matmul(
                            accs[ch][:], wfc_fm[:, kb, :],
                            h2_full[:, kb, 512 * c4:512 * (c4 + 1)],
                            start=(kb == 0), stop=False)
                for ch in range(2):
                    c4 = 2 * sh + ch
                    sl = slice(512 * c4, 512 * (c4 + 1))
                    hsl = slice(512 * ch, 512 * (ch + 1))
                    nc.tensor.matmul(accs[ch][:],
                                     wfcsum[0:1, 128 * fm:128 * (fm + 1)],
                                     nmu2_row[0:1, sl], start=False, stop=True)
                    pre = ev.tile([128, 512], f32, tag="ev_a")
                    nc.vector.tensor_mul(pre[:], accs[ch][:], s2_b[:, sl])
                    if with_biases:
                        nc.scalar.activation(out=gT[:, fm, hsl], in_=pre[:],
                                             func=AF.Gelu,
                                             bias=bfc_sb[:, fm, 0:1])
                    else:
                        nc.scalar.activation(out=gT[:, fm, hsl], in_=pre[:],
                                             func=AF.Gelu)
            for pm in range(KB):
                wproj_pm = stream.tile([128, MF, 128], f32, tag="ctx_kb")
                nc.sync.dma_start(
                    wproj_pm[:], wproj_d[:, 128 * pm:128 * (pm + 1)]
                    .rearrange("(fk p) m -> p fk m", p=128))
                accs = [ps.tile([128, 512], f32, tag=PT[2 + ch],
                                name=f"pacc{ch}") for ch in range(2)]
                for fk in range(MF):
                    for ch in range(2):
                        nc.tensor.matmul(
                            accs[ch][:], wproj_pm[:, fk, :],
                            gT[:, fk, 512 * ch:512 * (ch + 1)],
                            start=(fk == 0), stop=(fk == MF - 1))
                q = pm // 2
                for ch in range(2):
                    c4 = 2 * sh + ch
                    pev = ev.tile([128, 512], f32, tag="ev_a")
                    nc.vector.tensor_copy(pev[:], accs[ch][:])
                    nc.sync.dma_start(
                        rs_in[q][128 * (pm % 2):128 * (pm % 2 + 1),
                                 512 * c4:512 * (c4 + 1)], pev[:])
                if sh == 1 and pm % 2 == 1:
                    nc.gpsimd.collective_compute(
                        "ReduceScatter", mybir.AluOpType.add,
                        replica_groups=GROUPS,
                        ins=[rs_in[q][:].opt()], outs=[rs_out[q][:].opt()])
        for q in range(4):
            jr = slice(64 * (q % 2), 64 * (q % 2) + 64)
            fin_raw = stream.tile([128, S], f32, tag="ctx_kb",
                                  name=f"fin_raw{q}")
            nc.sync.dma_start(fin_raw[jr, :], rs_out[q][:])
            fin = stream.tile([128, S], f32, tag="ctx_kb", name=f"fin{q}")
            nc.vector.tensor_add(fin[jr, :], fin_raw[jr, :],
                                 h2s[jr, q // 2, :])
            if with_biases:
                nc.vector.tensor_scalar_add(fin[jr, :], fin[jr, :],
                                            bproj_sb[jr, q // 2, 0:1])
            nc.sync.dma_start(out_d[64 * q:64 * (q + 1), :], fin[jr, :])

    nc.compile()
    return nc


def _prep_inputs(x, ln1_g, ln1_b, w_qkv, b_qkv, w_ao, b_ao,
                 ln2_g, ln2_b, w_fc, b_fc, w_proj, b_proj):
    """Host-side sharding + LN folding. Returns (in_maps, with_biases)."""
    f = np.float32
    x = np.asarray(x, f)
    wqkv_eff = np.asarray(ln1_g, f)[:, None] * np.asarray(w_qkv, f)
    bqkv_eff = np.asarray(ln1_b, f) @ np.asarray(w_qkv, f) + np.asarray(b_qkv, f)
    wfc_eff = np.asarray(ln2_g, f)[:, None] * np.asarray(w_fc, f)
    bfc_eff = np.asarray(ln2_b, f) @ np.asarray(w_fc, f) + np.asarray(b_fc, f)
    w_ao = np.asarray(w_ao, f)
    b_ao = np.asarray(b_ao, f)
    w_proj = np.asarray(w_proj, f)
    b_proj = np.asarray(b_proj, f)

    with_biases = bool(
        np.any(bqkv_eff) or np.any(b_ao) or np.any(bfc_eff) or np.any(b_proj))

    # ReduceScatter permutation: proj_partial row j = 256q + 64rho + i  <->
    # output feature 256rho + 64q + i
    perm = np.empty(D, np.int64)
    for q in range(4):
        for rho in range(RANKS):
            perm[256 * q + 64 * rho:256 * q + 64 * rho + 64] = \
                np.arange(256 * rho + 64 * q, 256 * rho + 64 * q + 64)

    in_maps = []
    for c in range(NCORES):
        g, r = divmod(c, RANKS)
        xT = np.ascontiguousarray(x[g].T)                      # [D, S]
        xselT = np.ascontiguousarray(xT[DR * r:DR * (r + 1)])  # [DR, S]
        cols = np.concatenate([
            np.arange(part * H * DH + h * DH, part * H * DH + (h + 1) * DH)
            for part in range(3)
            for h in range(HLOC * r, HLOC * (r + 1))])
        wqkv_c = np.ascontiguousarray(wqkv_eff[:, cols])       # [D, 768]
        m = {
            "xT": xT,
            "xselT": xselT,
            "wqkv": wqkv_c,
            "wqkvsum": wqkv_c.sum(axis=0, keepdims=True),
            "wao": np.ascontiguousarray(w_ao[:, DR * r:DR * (r + 1)]),
            "wfc": np.ascontiguousarray(wfc_eff[:, FR * r:FR * (r + 1)]),
            "wfcsum": wfc_eff[:, FR * r:FR * (r + 1)].sum(axis=0, keepdims=True),
            "wproj": np.ascontiguousarray(w_proj[FR * r:FR * (r + 1)][:, perm]),
        }
        if with_biases:
            m["bqkv"] = np.ascontiguousarray(bqkv_eff[cols][:, None])
            m["bao"] = np.ascontiguousarray(b_ao[DR * r:DR * (r + 1)][:, None])
            m["bfc"] = np.ascontiguousarray(bfc_eff[FR * r:FR * (r + 1)][:, None])
            m["bproj"] = np.ascontiguousarray(
                b_proj[DR * r:DR * (r + 1)][:, None])
        in_maps.append(m)
    return in_maps, with_biases


def run_spmd(nc, in_maps, **kw):
    from concourse.bass_utils import run_bass_kernel_spmd
    return run_bass_kernel_spmd(nc, in_maps, list(range(NCORES)), **kw)


def kernel(**inputs):
    in_maps, with_biases = _prep_inputs(**inputs)
    if with_biases not in _CACHE:
        _CACHE[with_biases] = _build(with_biases)
    res = run_spmd(_CACHE[with_biases], in_maps)
    out = np.empty((B, S, D), np.float32)
    for c in range(NCORES):
        g, r = divmod(c, RANKS)
        out[g, :, DR * r:DR * (r + 1)] = res.results[c]["out"].T
    return out
